# revision 7
# baseline (speedup 1.0000x reference)
"""Trainium2 Bass kernel for nn_Bond2BondLayer (gnn_message_passing).

Strategy
--------
Triplets are sharded across 8 cores by the edge-range of index_ji (the
segment-softmax key).  Each core owns edges [c*25000, (c+1)*25000) and
receives exactly the triplets targeting those edges, sorted by edge id.
The segment softmax is then fully core-local (no collectives):

  v[e] = (sum_t exp(att_t) * feat_kj_t) / (sum_t exp(att_t)) + k_b

i.e. one pass accumulating unnormalised numerator [128, E_c] and
denominator [8, E_c] in SBUF, normalising per edge at the end.

Host-side prep (part of sharding): sort/permute triplets, compute the
angle class (pure input-derived geometry), pack per-tile index/rel/dd
metadata, and bake the per-tile accumulator windows (compile-time
constants shared by all cores -> single SPMD program).
"""

import numpy as np

import concourse.bass as bass
import concourse.bacc as bacc
import concourse.mybir as mybir
from concourse.bass_utils import run_bass_kernel_spmd
from concourse.masks import make_identity
from concourse.tile import TileContext

F32 = mybir.dt.float32
I32 = mybir.dt.int32

HIDDEN = 128
NUM_HEAD = 8
OUT_FEAT = 16
CLASS_NUM = 6
PI_CONST = 3.1415926
N_NODES = 50000
N_EDGES = 200000
N_TRIP = 500000
NCORES = 8
EC = N_EDGES // NCORES            # 25000 edges per core
EPAD = 25088                      # EC padded to a multiple of 512
SPLITS = [(0, 12288), (12288, EPAD)]   # accumulator windows (local edges)
ST = 512                          # triplets per supertile (4 tiles of 128)
INF = 1 << 60


# --------------------------------------------------------------------------
# host-side preparation
# --------------------------------------------------------------------------

def _host_prep(pos, dist_decay, bond_embedding, index_kj, index_ji,
               idx_i, idx_j, idx_k, params):
    p = params

    # ---- angle class (pure input geometry, no params) ----
    pi_ = pos[idx_i]
    dj = pos[idx_j] - pi_
    dk = pos[idx_k] - pi_
    a = np.einsum('td,td->t', dj, dk)
    b = np.linalg.norm(np.cross(dj, dk), axis=-1)
    angle = np.arctan2(b, a)
    cls = (angle / (PI_CONST / CLASS_NUM)).astype(np.int32)
    np.clip(cls, 0, CLASS_NUM - 1, out=cls)

    dd_t = dist_decay[index_kj].astype(np.float32)

    # ---- 6-row angle-embedding table through its 3 dense layers ----
    ae = np.asarray(p['ang_emb'], np.float32)
    ae = np.maximum(ae @ p['ang_in_w'] + p['ang_in_b'], 0.0)
    ae = np.maximum(ae @ p['ang2_w'] + p['ang2_b'], 0.0)
    ae = np.maximum(ae @ p['ang1_w'] + p['ang1_b'], 0.0)
    ae6p = (ae + p['k_b'] + p['q_b']).astype(np.float32)      # bias fold

    # ---- attention weight as a [128, 8] block-diagonal matrix ----
    attn = np.asarray(p['attn'], np.float32).reshape(NUM_HEAD, OUT_FEAT)
    watt = np.zeros((HIDDEN, NUM_HEAD), np.float32)
    for h in range(NUM_HEAD):
        watt[h * OUT_FEAT:(h + 1) * OUT_FEAT, h] = attn[h]

    # ---- head replicator [8, 128] ----
    r16 = np.zeros((NUM_HEAD, HIDDEN), np.float32)
    for h in range(NUM_HEAD):
        r16[h, h * OUT_FEAT:(h + 1) * OUT_FEAT] = 1.0

    # ---- k_b folded into lin1 bias ----
    lin1b = (np.asarray(p['lin1_b'], np.float32)
             + np.asarray(p['k_b'], np.float32) @ np.asarray(p['lin1_w'], np.float32))

    consts = dict(
        kw=np.ascontiguousarray(p['k_w'], dtype=np.float32),
        qw=np.ascontiguousarray(p['q_w'], dtype=np.float32),
        ae6p=ae6p,
        watt=watt,
        r16=r16,
        lin1w=np.ascontiguousarray(p['lin1_w'], dtype=np.float32),
        lin2w=np.ascontiguousarray(p['lin2_w'], dtype=np.float32),
        r1aw=np.ascontiguousarray(p['r1a_w'], dtype=np.float32),
        r1bw=np.ascontiguousarray(p['r1b_w'], dtype=np.float32),
        r2aw=np.ascontiguousarray(p['r2a_w'], dtype=np.float32),
        r2bw=np.ascontiguousarray(p['r2b_w'], dtype=np.float32),
        biases=np.stack([lin1b,
                         np.asarray(p['lin2_b'], np.float32),
                         np.asarray(p['r1a_b'], np.float32),
                         np.asarray(p['r1b_b'], np.float32),
                         np.asarray(p['r2a_b'], np.float32),
                         np.asarray(p['r2b_b'], np.float32)], axis=1),  # [128, 6]
    )

    # ---- sort triplets by target edge, split by core edge-range ----
    order = np.argsort(index_ji, kind='stable')
    sji = index_ji[order]
    starts = np.searchsorted(sji, np.arange(NCORES) * EC).astype(np.int64)
    ends = np.searchsorted(sji, (np.arange(NCORES) + 1) * EC).astype(np.int64)

    # ---- unified greedy tiling across the 8 cores ----
    ptr = starts.copy()
    tiles = []          # (split_idx, base) in local edge coords
    tile_fill = []      # per tile: list of (core, sorted-range lo, hi)
    for si, (slo, shi) in enumerate(SPLITS):
        while True:
            pend = [sji[ptr[c]] - c * EC if ptr[c] < ends[c] else INF
                    for c in range(NCORES)]
            base = min(pend)
            if base >= shi:
                break
            wh = min(base + 128, shi)
            fills = []
            for c in range(NCORES):
                if ptr[c] >= ends[c] or pend[c] >= wh:
                    fills.append((c, ptr[c], ptr[c]))
                    continue
                hi = ptr[c] + np.searchsorted(sji[ptr[c]:ends[c]], wh + c * EC)
                n = min(128, hi - ptr[c])
                fills.append((c, ptr[c], ptr[c] + n))
                ptr[c] += n
            tiles.append((si, int(base)))
            tile_fill.append(fills)
        # pad split's tile count to a multiple of 4
        while len([t for t in tiles if t[0] == si]) % 4 != 0:
            tiles.append((si, slo))
            tile_fill.append([(c, 0, 0) for c in range(NCORES)])

    ntiles = len(tiles)
    nsup = ntiles // 4
    nsup_split = [len([t for t in tiles if t[0] == si]) // 4
                  for si in range(len(SPLITS))]
    bases = [b for (_, b) in tiles]

    # ---- pack per-core per-tile arrays ----
    rel_neg = np.float32(-1.0).view(np.int32)
    meta = np.zeros((NCORES, nsup, 128, 16), np.int32)
    meta[:, :, :, 8:12] = rel_neg           # rel = -1 -> dummy slot
    oh6 = np.zeros((NCORES, nsup, 6, ST), np.float32)

    f32i = lambda x: x.astype(np.float32).view(np.int32)
    for k, ((si, bse), fills) in enumerate(zip(tiles, tile_fill)):
        s, b = k // 4, k % 4
        for (c, lo, hi) in fills:
            n = hi - lo
            if n == 0:
                continue
            trip = order[lo:hi]
            meta[c, s, :n, 0 + b] = index_kj[trip]
            meta[c, s, :n, 4 + b] = index_ji[trip]
            meta[c, s, :n, 8 + b] = f32i(sji[lo:hi] - c * EC - bse)
            meta[c, s, :n, 12 + b] = f32i(dd_t[trip])
            oh6[c, s, cls[trip], 128 * b + np.arange(n)] = 1.0

    # ---- per-core transposed bond slices ----
    bondT = np.zeros((NCORES, HIDDEN, EPAD), np.float32)
    for c in range(NCORES):
        bondT[c, :, :EC] = bond_embedding[c * EC:(c + 1) * EC].T

    plan = dict(nsup=nsup, nsup_split=nsup_split, bases=bases,
                splits=SPLITS, epad=EPAD)
    data = dict(meta=meta, oh6=oh6, bondT=bondT,
                bond=np.ascontiguousarray(bond_embedding, dtype=np.float32),
                consts=consts)
    return plan, data


# --------------------------------------------------------------------------
# device program
# --------------------------------------------------------------------------

def _build_program(plan, n_rep=1):
    nsup = plan['nsup']
    nsup_split = plan['nsup_split']
    bases = plan['bases']
    splits = plan['splits']
    epad = plan['epad']

    nc = bacc.Bacc("TRN2", target_bir_lowering=False, debug=False,
                   num_devices=NCORES)
    dt = lambda n, s, k='ExternalInput': nc.dram_tensor(n, s, F32, kind=k).ap()
    bond = dt("bond", [N_EDGES, HIDDEN])
    bondT = dt("bondT", [HIDDEN, epad])
    meta_d = nc.dram_tensor("meta", [nsup, 128, 16], I32,
                            kind="ExternalInput").ap()
    oh6_d = dt("oh6", [nsup, 6, ST])
    kw = dt("kw", [HIDDEN, HIDDEN]); qw = dt("qw", [HIDDEN, HIDDEN])
    ae6p_d = dt("ae6p", [CLASS_NUM, HIDDEN])
    watt_d = dt("watt", [HIDDEN, NUM_HEAD])
    r16_d = dt("r16", [NUM_HEAD, HIDDEN])
    lw = {n: dt(n, [HIDDEN, HIDDEN])
          for n in ("lin1w", "lin2w", "r1aw", "r1bw", "r2aw", "r2bw")}
    biases_d = dt("biases", [HIDDEN, 6])
    outT = dt("outT", [HIDDEN, epad], 'ExternalOutput')

    W0 = splits[0][1] - splits[0][0]
    W1 = splits[1][1] - splits[1][0]
    WMAX = max(W0, W1)

    from contextlib import ExitStack
    with TileContext(nc) as tc, ExitStack() as stk:
        cp = stk.enter_context(tc.tile_pool(name="const", bufs=1))
        ap = stk.enter_context(tc.tile_pool(name="acc", bufs=1))
        wp = stk.enter_context(tc.tile_pool(name="work", bufs=2))
        mp = stk.enter_context(tc.tile_pool(name="meta", bufs=3))
        pp = stk.enter_context(tc.tile_pool(name="psum", bufs=3, space="PSUM"))
        fp = stk.enter_context(tc.tile_pool(name="fpsum", bufs=2, space="PSUM"))
        if True:
            # ---------- constants into SBUF ----------
            c_kw = cp.tile([HIDDEN, HIDDEN], F32)
            c_qw = cp.tile([HIDDEN, HIDDEN], F32)
            c_ae = cp.tile([CLASS_NUM, HIDDEN], F32)
            c_wa = cp.tile([HIDDEN, NUM_HEAD], F32)
            c_r16 = cp.tile([NUM_HEAD, HIDDEN], F32)
            c_lw = {n: cp.tile([HIDDEN, HIDDEN], F32, name=f"c_{n}") for n in lw}
            c_bias = cp.tile([HIDDEN, 6], F32)
            c_id = cp.tile([HIDDEN, HIDDEN], F32)
            nc.sync.dma_start(out=c_kw[:], in_=kw[:, :])
            nc.sync.dma_start(out=c_qw[:], in_=qw[:, :])
            nc.sync.dma_start(out=c_ae[:], in_=ae6p_d[:, :])
            nc.sync.dma_start(out=c_wa[:], in_=watt_d[:, :])
            nc.sync.dma_start(out=c_r16[:], in_=r16_d[:, :])
            for n in lw:
                nc.sync.dma_start(out=c_lw[n][:], in_=lw[n][:, :])
            nc.sync.dma_start(out=c_bias[:], in_=biases_d[:, :])
            make_identity(nc, c_id[:])
            c_iota_i = cp.tile([HIDDEN, HIDDEN], I32)
            c_iota = cp.tile([HIDDEN, HIDDEN], F32)
            nc.gpsimd.iota(c_iota_i[:], pattern=[[1, HIDDEN]],
                           channel_multiplier=0)
            nc.vector.tensor_copy(c_iota[:], c_iota_i[:])

            acc_n = ap.tile([HIDDEN, WMAX + 128], F32)
            acc_d = ap.tile([NUM_HEAD, WMAX + 128], F32)

            k0 = 0
            with tc.For_i(0, n_rep, 1):
                for si, (slo, shi) in enumerate(splits):
                    if si == 0:
                        k0 = 0
                    W = shi - slo
                    nc.vector.memset(acc_n[:, :W + 128], 0.0)
                    nc.vector.memset(acc_d[:, :W + 128], 1e-30)

                    for s in range(k0, k0 + nsup_split[si]):
                        mt = mp.tile([128, 16], I32, tag="mt")
                        nc.sync.dma_start(out=mt[:], in_=meta_d[s, :, :])
                        oh = mp.tile([CLASS_NUM, ST], F32, tag="oh")
                        nc.sync.dma_start(out=oh[:], in_=oh6_d[s, :, :])

                        gkj = wp.tile([128, ST], F32, tag="gkj")
                        gji = wp.tile([128, ST], F32, tag="gji")
                        for b in range(4):
                            nc.gpsimd.indirect_dma_start(
                                out=gkj[:, 128 * b:128 * (b + 1)],
                                out_offset=None, in_=bond[:, :],
                                in_offset=bass.IndirectOffsetOnAxis(
                                    ap=mt[:, b:b + 1], axis=0))
                            nc.gpsimd.indirect_dma_start(
                                out=gji[:, 128 * b:128 * (b + 1)],
                                out_offset=None, in_=bond[:, :],
                                in_offset=bass.IndirectOffsetOnAxis(
                                    ap=mt[:, 4 + b:5 + b], axis=0))

                        # transpose gathers: [t,f] -> [f,t]
                        tkj_p = pp.tile([128, 1024], F32, tag="big")
                        tji_p = pp.tile([128, 1024], F32, tag="big")
                        for b in range(4):
                            sl = slice(128 * b, 128 * (b + 1))
                            nc.tensor.transpose(out=tkj_p[:, sl],
                                                in_=gkj[:, sl], identity=c_id[:])
                            nc.tensor.transpose(out=tji_p[:, sl],
                                                in_=gji[:, sl], identity=c_id[:])
                        bkjT = wp.tile([128, ST], F32, tag="bkjT")
                        bjiT = wp.tile([128, ST], F32, tag="bjiT")
                        nc.vector.tensor_copy(bkjT[:], tkj_p[:, :ST])
                        nc.scalar.copy(bjiT[:], tji_p[:, :ST])

                        # feat = lrelu(K + Q + AE) in [d, t]
                        fps = fp.tile([HIDDEN, ST], F32, tag="feat")
                        nc.tensor.matmul(fps[:], lhsT=c_kw[:], rhs=bkjT[:],
                                         start=True, stop=False)
                        nc.tensor.matmul(fps[:], lhsT=c_qw[:], rhs=bjiT[:],
                                         start=False, stop=False)
                        nc.tensor.matmul(fps[:], lhsT=c_ae[:], rhs=oh[:],
                                         start=False, stop=True)
                        feat = wp.tile([HIDDEN, ST], F32, tag="feat_s")
                        nc.scalar.activation(feat[:], fps[:],
                                             mybir.ActivationFunctionType.Lrelu,
                                             alpha=0.01)

                        # fk_td [t, d] and att [t, 8] per sub-tile
                        fka = pp.tile([128, 1024], F32, tag="big")
                        ad4 = wp.tile([128, 4 * NUM_HEAD], F32, tag="ad4")
                        for b in range(4):
                            sl = slice(128 * b, 128 * (b + 1))
                            nc.tensor.matmul(fka[:, sl], lhsT=bkjT[:, sl],
                                             rhs=c_kw[:], start=True, stop=True)
                            asl = slice(ST + 8 * b, ST + 8 * (b + 1))
                            nc.tensor.matmul(fka[:, asl], lhsT=feat[:, sl],
                                             rhs=c_wa[:], start=True, stop=True)
                            dd_col = mt[:, 12 + b:13 + b].bitcast(F32)
                            nc.scalar.activation(
                                ad4[:, 8 * b:8 * (b + 1)], fka[:, asl],
                                mybir.ActivationFunctionType.Exp, bias=dd_col)

                        # one-hot O, weighted C, delta matmuls, accumulate
                        dl = pp.tile([128, 1024], F32, tag="big")
                        for b in range(4):
                            k = (s - k0) * 4 + b + sum(
                                4 * nsup_split[x] for x in range(si))
                            base = bases[k] - slo
                            rel_col = mt[:, 8 + b:9 + b].bitcast(F32)
                            O = wp.tile([128, 128], F32, tag="O")
                            nc.vector.tensor_tensor(
                                out=O[:], in0=rel_col.to_broadcast([128, 128]),
                                in1=c_iota[:], op=mybir.AluOpType.is_equal)
                            C = wp.tile([128, HIDDEN], F32, tag="C")
                            nc.vector.tensor_tensor(
                                out=C[:].rearrange("p (h w) -> p h w", h=NUM_HEAD),
                                in0=fka[:, 128 * b:128 * (b + 1)].rearrange(
                                    "p (h w) -> p h w", h=NUM_HEAD),
                                in1=ad4[:, 8 * b:8 * (b + 1)][:, :, None]
                                    .to_broadcast([128, NUM_HEAD, OUT_FEAT]),
                                op=mybir.AluOpType.mult)
                            nsl = slice(256 * b, 256 * b + 128)
                            dsl = slice(256 * b + 128, 256 * b + 256)
                            nc.tensor.matmul(dl[:, nsl], lhsT=C[:], rhs=O[:],
                                             start=True, stop=True)
                            nc.tensor.matmul(dl[:NUM_HEAD, dsl],
                                             lhsT=ad4[:, 8 * b:8 * (b + 1)],
                                             rhs=O[:], start=True, stop=True)
                            nc.vector.tensor_add(
                                out=acc_n[:, base:base + 128],
                                in0=acc_n[:, base:base + 128], in1=dl[:, nsl])
                            nc.vector.tensor_add(
                                out=acc_d[:, base:base + 128],
                                in0=acc_d[:, base:base + 128],
                                in1=dl[:NUM_HEAD, dsl])
                    k0 += nsup_split[si]

                    # ---------- normalise + edge MLP for this split ----------
                    for m in range(W // ST):
                        lo = m * ST
                        gsl = slice(slo + lo, slo + lo + ST)
                        rd = wp.tile([NUM_HEAD, ST], F32, tag="rd")
                        nc.vector.reciprocal(rd[:], acc_d[:, lo:lo + ST])
                        rps = fp.tile([HIDDEN, ST], F32, tag="feat")
                        nc.tensor.matmul(rps[:], lhsT=c_r16[:], rhs=rd[:],
                                         start=True, stop=True)
                        v = wp.tile([HIDDEN, ST], F32, tag="v")
                        nc.vector.tensor_mul(v[:], acc_n[:, lo:lo + ST], rps[:])

                        def dense(x_s, wname, bcol, act, tag):
                            ps = pp.tile([128, 1024], F32, tag="big")
                            nc.tensor.matmul(ps[:, :ST], lhsT=c_lw[wname][:],
                                             rhs=x_s[:], start=True, stop=True)
                            o = wp.tile([HIDDEN, ST], F32, tag=tag)
                            bc = c_bias[:, bcol:bcol + 1]
                            if act == 'relu_act':
                                nc.scalar.activation(
                                    o[:], ps[:, :ST],
                                    mybir.ActivationFunctionType.Relu, bias=bc)
                            elif act == 'relu_dve':
                                nc.vector.tensor_scalar(
                                    out=o[:], in0=ps[:, :ST], scalar1=bc,
                                    scalar2=0.0, op0=mybir.AluOpType.add,
                                    op1=mybir.AluOpType.max)
                            else:
                                nc.vector.tensor_scalar(
                                    out=o[:], in0=ps[:, :ST], scalar1=bc,
                                    scalar2=None, op0=mybir.AluOpType.add)
                            return o

                        bt = wp.tile([HIDDEN, ST], F32, tag="bt")
                        nc.sync.dma_start(out=bt[:], in_=bondT[:, gsl])
                        h1 = dense(v, "lin1w", 0, 'relu_act', "h1")
                        h2 = dense(h1, "lin2w", 1, 'add', "h2")
                        he = wp.tile([HIDDEN, ST], F32, tag="he")
                        nc.vector.tensor_add(he[:], h2[:], bt[:])
                        t1 = dense(he, "r1aw", 2, 'relu_act', "t1")
                        t2 = dense(t1, "r1bw", 3, 'relu_dve', "t2")
                        he2 = wp.tile([HIDDEN, ST], F32, tag="he2")
                        nc.vector.tensor_add(he2[:], he[:], t2[:])
                        t3 = dense(he2, "r2aw", 4, 'relu_act', "t3")
                        t4 = dense(t3, "r2bw", 5, 'relu_dve', "t4")
                        ot = wp.tile([HIDDEN, ST], F32, tag="ot")
                        nc.vector.tensor_add(ot[:], he2[:], t4[:])
                        nc.sync.dma_start(out=outT[:, gsl], in_=ot[:])
    nc.compile()
    return nc


# --------------------------------------------------------------------------
# entry point
# --------------------------------------------------------------------------

def kernel(pos, dist_decay, bond_embedding, index_kj, index_ji,
           idx_i, idx_j, idx_k, params, _n_rep=1, _prebuilt=None):
    plan, data = _host_prep(pos, dist_decay, bond_embedding, index_kj,
                            index_ji, idx_i, idx_j, idx_k, params)
    nc = _prebuilt if _prebuilt is not None else _build_program(plan, _n_rep)

    c = data['consts']
    in_maps = []
    for core in range(NCORES):
        m = dict(bond=data['bond'], bondT=data['bondT'][core],
                 meta=data['meta'][core], oh6=data['oh6'][core],
                 kw=c['kw'], qw=c['qw'], ae6p=c['ae6p'], watt=c['watt'],
                 r16=c['r16'], lin1w=c['lin1w'], lin2w=c['lin2w'],
                 r1aw=c['r1aw'], r1bw=c['r1bw'], r2aw=c['r2aw'],
                 r2bw=c['r2bw'], biases=c['biases'])
        in_maps.append(m)

    res = run_bass_kernel_spmd(nc, in_maps, core_ids=list(range(NCORES)))
    out = np.empty((N_EDGES, HIDDEN), np.float32)
    for core in range(NCORES):
        out[core * EC:(core + 1) * EC] = res.results[core]["outT"][:, :EC].T
    return out
